# revision 41
# baseline (speedup 1.0000x reference)
"""Trainium2 Bass kernel for DiscreteDeltaThetaGammaLayer.

Coupled Kuramoto-oscillator recurrence:
  phase0 = (x @ W_phase.T) mod 2pi ; amp0 = max(|x @ W_amp.T|, eps)
  32 steps of: intra-band Kuramoto coupling (phase), PAC amplitude modulation
  output: final amp  (4096, 352) f32

Strategy (8 NeuronCores, data-parallel over batch, 512 rows/core):
  - State held transposed [128 osc partitions x batch free]. Oscillators
    permuted into chunks: c0 = delta(32)+theta(64)+pad(32), c1/c2 = gamma
    halves. Zero blocks of K.T are skipped.
  - Phase kept wrapped in [-pi, pi] (ACT Sin LUT accurate there only);
    cos(phi) = sin(pi/2 - |phi|). The |phi| pass and the t = phi + cos*v
    add run on the Pool engine (all-SBUF; GPSIMD cannot read PSUM), the
    [cos|sin]*[v|u] product and the fused WRAP_SUB phase update
    wrap((t - sin*u) + dt*omega) run on the Vector engine.
  - Per-core batch split into two 256-col streams so the recurrence
    pipelines across engines, and every f32r matmul has >=256 moving cols
    (full PE rate).
  - Band sums (PE matmuls vs band-indicator) write straight into a dedicated
    PSUM bank laid out [step, q, (Sd St Cd Ct)]; one DMA at the end. Host
    reconstructs amp exactly: f_k, prefix products P, running min m,
    amp = max(amp0*P, eps*P/m).
  - Projections are full-core (512 cols, f32r full rate) into PSUM reused by
    the per-stream vu tiles.
"""

import math
import os
import sys

sys.path.insert(0, "/opt/trn_rl_repo")

import numpy as np

# ---- problem constants (module hyperparameters) ----
N_DELTA, N_THETA, N_GAMMA = 32, 64, 256
N_TOTAL = 352
N_DIMS = 1024
BATCH = 4096
N_STEPS = 32
DT = 0.01
PAC = 0.3
EPS = 1e-6
TWO_PI = 2.0 * math.pi
PI = math.pi

N_CORES = 8
BL = BATCH // N_CORES          # 512 batch rows per core
BHS = [256, 256]               # two independent streams
OFFS = [0, 256]
NH = len(BHS)
P = 128
NCH = 3                        # oscillator chunks (3*128 = 384 >= 352)
CHUNK_REAL = [96, 128, 128]
KD = N_DIMS // P               # 8 contraction chunks for the projections

LAST_EXEC_NS = None
_COMPILED = {}
_WRAP_SUB = None


def _osc_perm():
    """orig oscillator index for each (chunk, partition); -1 for pads."""
    perm = -np.ones((NCH, P), dtype=np.int64)
    perm[0, :96] = np.arange(96)           # delta + theta
    perm[1, :] = 96 + np.arange(128)       # gamma 0:128
    perm[2, :] = 224 + np.arange(128)      # gamma 128:256
    return perm


def _get_wrap_sub():
    """Custom DVE op: out = wrap((in0 - in1) + s0) into [-s1, s1], period imm2."""
    global _WRAP_SUB
    if _WRAP_SUB is not None:
        return _WRAP_SUB
    from concourse.dve_spec import C0, C1, C2, Spec, Src0, Src1, lower
    from concourse.dve_uop import DveOpSpec
    import concourse.dve_ops as dvo

    def _ref(in0, in1, s0, s1, imm2):
        y = (in0 - in1) + s0
        return (y + imm2 * (y < -s1).astype(np.float32)
                - imm2 * (y > s1).astype(np.float32)).astype(np.float32)

    _y = (Src0 - Src1) + C0
    spec = Spec(body=(_y + C2 * (_y < -C1)) - C2 * (_y > C1), reference=_ref)
    shas = {}
    for ver in ("v3", "v4"):
        tmp = DveOpSpec(name="WRAP_SUB_KERNEL", opcode=31,
                        uops=lower(spec, ver=ver), rd1_en=True)
        shas[ver] = tmp.sha(ver)
    op = dvo.DveOp("WRAP_SUB_KERNEL", spec, subdim=False, uops_sha=shas)
    dvo.OPS.append(op)
    dvo.CUSTOM_DVE_SPECS[op.name] = op.spec
    dvo._SUB_OPCODE_FOR_NAME[op.name] = dvo._CUSTOM_DVE_ROW_BASE + len(dvo.OPS) - 1
    _WRAP_SUB = op
    return op


def _build_program(nz_pairs, merge_g=False):
    import concourse.bass as bass
    import concourse.tile as tile
    from concourse import bacc, mybir

    wrap_sub = _get_wrap_sub()

    f32 = mybir.dt.float32
    f32r = mybir.dt.float32r
    bf16 = mybir.dt.bfloat16
    AF = mybir.ActivationFunctionType
    ALU = mybir.AluOpType

    nc = bacc.Bacc("TRN2", target_bir_lowering=False, debug=False)

    # ---- DRAM I/O ----
    xT = nc.dram_tensor("xT", [N_DIMS, BL], f32r, kind="ExternalInput").ap()
    wpT = nc.dram_tensor("wpT", [N_DIMS, NCH * P], f32r, kind="ExternalInput").ap()
    waT = nc.dram_tensor("waT", [N_DIMS, NCH * P], f32r, kind="ExternalInput").ap()
    kT = nc.dram_tensor("kT", [NCH * P, NCH * P], f32, kind="ExternalInput").ap()
    dtw = nc.dram_tensor("dtw", [P, NCH], f32, kind="ExternalInput").ap()
    wband = nc.dram_tensor("wband", [P, 2], f32, kind="ExternalInput").ap()

    amp0_out = nc.dram_tensor("amp0", [P, NCH * BL], f32, kind="ExternalOutput").ap()
    # bsums layout: col = step*16 + q*4 + (Sd St Cd Ct), q = batch block (q*128)
    bs_out = nc.dram_tensor(
        "bsums", [P, N_STEPS * 16], f32, kind="ExternalOutput"
    ).ap()

    with tile.TileContext(nc) as tc:
        with (
            tc.tile_pool(name="state", bufs=1) as state_pool,
            tc.tile_pool(name="weights", bufs=1) as wpool,
            tc.tile_pool(name="work", bufs=3) as work,
            tc.tile_pool(name="psum", bufs=1, space="PSUM") as psum,
        ):
            # ---- persistent constants ----
            dtw_sb = wpool.tile([P, NCH], f32, tag="dtw")
            nc.gpsimd.dma_start(dtw_sb[:], dtw[:])
            pihalf = wpool.tile([P, 1], f32, tag="pihalf")
            nc.vector.memset(pihalf[:], PI / 2.0)
            wband_f = wpool.tile([P, 2], f32, tag="wband_f")
            nc.gpsimd.dma_start(wband_f[:], wband[:])
            wband_sb = wpool.tile([P, 2], bf16, tag="wband")
            nc.vector.tensor_copy(wband_sb[:], wband_f[:])

            kt_sb = {}
            kt_ng = {}
            for (jc, ic) in nz_pairs:
                tf = work.tile([P, P], f32, tag="ktld")
                nc.gpsimd.dma_start(tf[:], kT[jc * P:(jc + 1) * P, ic * P:(ic + 1) * P])
                t = wpool.tile([P, P], bf16, tag=f"kt_{jc}_{ic}")
                nc.vector.tensor_copy(t[:], tf[:])
                kt_sb[(jc, ic)] = t
                tn = wpool.tile([P, P], bf16, tag=f"ktn_{jc}_{ic}")
                nc.vector.tensor_scalar_mul(tn[:], tf[:], -1.0)
                kt_ng[(jc, ic)] = tn

            # ---- big input loads ----
            # phase-path first: only it gates the recurrence; the amp
            # projection can lag and overlap the first steps.
            xk = []
            wk_all = {}
            for k in range(KD):
                t = wpool.tile([P, BL], f32r, tag=f"x_{k}")
                eng = nc.gpsimd if k % 2 == 0 else nc.sync
                eng.dma_start(t[:], xT[k * P:(k + 1) * P, :])
                xk.append(t)
                t = wpool.tile([P, NCH * P], f32r, tag=f"w0_{k}")
                nc.sync.dma_start(t[:], wpT[k * P:(k + 1) * P, :])
                wk_all[(0, k)] = t
            for k in range(KD):
                t = wpool.tile([P, NCH * P], f32r, tag=f"w1_{k}")
                nc.sync.dma_start(t[:], waT[k * P:(k + 1) * P, :])
                wk_all[(1, k)] = t

            # ---- per-stream state ----
            # vu halves are separate PSUM tiles so a product TT only waits on
            # its own half's coupling matmuls (dependency tracking is
            # tile-granular)
            WH = NCH * BHS[0]  # 768
            phi = []
            for h in range(NH):
                phi.append(state_pool.tile([P, WH], bf16, tag=f"phi{h}",
                                           name=f"phi{h}"))
            # TILE-A = [u(768) | v_chunk0(256)] (2 banks), TILE-B =
            # [v_c1 | v_c2] (1 bank) per stream: 7 banks total with bs.
            # The critical sin*u product reads TILE-A, whose only
            # sin-dependent writer (the single v_c0 matmul) finishes early.
            # The v half uses negated K so the wrap op (which subtracts its
            # in1) effectively adds cos*v.
            vu_a, vu_b = [], []
            for h in range(NH):
                vu_a.append(psum.tile([P, WH + BHS[0]], f32, tag=f"vua{h}",
                                      name=f"vua{h}"))
                vu_b.append(psum.tile([P, 2 * BHS[0]], f32, tag=f"vub{h}",
                                      name=f"vub{h}"))
            bs = psum.tile([P, N_STEPS * 16], f32, tag="bs", name="bs")

            # ---- initial projections: per (stream, chunk) 256-col pieces
            # laid out to match the per-stream vu tiles (f32r full rate) ----
            # per-stream: phase proj + wrap, then amp proj + abs. Stream 1's
            # projections queue behind stream 0's on the PE, which skews the
            # two recurrences by roughly half a step cycle - that anti-phase
            # is what lets the engine queues interleave the streams.
            ab = work.tile([P, NCH * BL], f32, tag="abs0", name="abs0")
            for h in range(NH):
                bh = BHS[h]
                for c in range(NCH):
                    acc = vu_a[h][:, c * bh:(c + 1) * bh]
                    for k in range(KD):
                        nc.tensor.matmul(
                            acc, wk_all[(0, k)][:, c * P:(c + 1) * P],
                            xk[k][:, OFFS[h]:OFFS[h] + bh],
                            start=(k == 0), stop=(k == KD - 1),
                        )
                nc.vector.add_range_wrap(phi[h][:], vu_a[h][:, 0:WH], 0.0, PI,
                                         TWO_PI)
                adst = [vu_b[h][:, 0:bh], vu_b[h][:, bh:2 * bh],
                        vu_a[h][:, WH:WH + bh]]
                for c in range(NCH):
                    for k in range(KD):
                        nc.tensor.matmul(
                            adst[c], wk_all[(1, k)][:, c * P:(c + 1) * P],
                            xk[k][:, OFFS[h]:OFFS[h] + bh],
                            start=(k == 0), stop=(k == KD - 1),
                        )
                for c in range(NCH):
                    nc.scalar.activation(
                        ab[:, c * BL + OFFS[h]:c * BL + OFFS[h] + bh],
                        adst[c], AF.Abs)
            nc.sync.dma_start(amp0_out[:], ab[:])

            # ---- the recurrence: NH independent streams, staged emission so
            # each engine queue always has the other stream's independent
            # work between dependent ops ----
            spans = [(0, 1), (1, 3)] if merge_g else [(0, 1), (1, 2), (2, 3)]
            for it in range(N_STEPS + 1):
                bh = BHS[0]
                wh = NCH * bh
                nq = bh // P
                cw = bh if it == N_STEPS else wh
                horder = list(range(NH))
                sin_t, cos_t, pa_t = {}, {}, {}
                for h in horder:
                    sin_t[h] = work.tile([P, wh], bf16, tag=f"css{h}",
                                         name=f"css{h}")
                    cos_t[h] = work.tile([P, wh], bf16, tag=f"csc{h}",
                                         name=f"csc{h}")
                    pa_t[h] = work.tile([P, wh], f32, tag=f"pa{h}",
                                        name=f"pa{h}")
                # trig: abs -> cos first (cos gates the u matmuls feeding
                # the critical sin*u product); sin runs while the PE does u
                for h in horder:
                    nc.scalar.activation(pa_t[h][:, 0:cw], phi[h][:, 0:cw],
                                         AF.Abs)
                    nc.scalar.activation(cos_t[h][:, 0:cw], pa_t[h][:, 0:cw],
                                         AF.Sin, bias=pihalf[:], scale=-1.0)
                    nc.scalar.activation(sin_t[h][:, 0:cw], phi[h][:, 0:cw],
                                         AF.Sin)

                # coupling: u = (dt*K) cos (plain) first, then the negated
                # v = -(dt*K) sin, v_c0 first so TILE-A completes early
                if it < N_STEPS:
                    for h in horder:
                        vdst = [vu_a[h][:, WH:WH + bh],
                                vu_b[h][:, 0:bh], vu_b[h][:, bh:2 * bh]]
                        for neg, src in ((False, cos_t[h]), (True, sin_t[h])):
                            for ic in range(NCH):
                                jcs = [jc for (jc, i2) in nz_pairs if i2 == ic]
                                if neg:
                                    dst = vdst[ic]
                                    ktu = kt_ng
                                else:
                                    dst = vu_a[h][:, ic * bh:(ic + 1) * bh]
                                    ktu = kt_sb
                                for n, jc in enumerate(jcs):
                                    nc.tensor.matmul(
                                        dst,
                                        ktu[(jc, ic)][:],
                                        src[:, jc * bh:(jc + 1) * bh],
                                        start=(n == 0),
                                        stop=(n == len(jcs) - 1),
                                    )

                # band sums -> bs[step, q]: (Sd St Cd Ct), q = h*nq + local
                # (off the critical path; emitted after the coupling)
                if it > 0:
                    for h in horder:
                        for q in range(nq):
                            col = (it - 1) * 16 + (h * nq + q) * 4
                            nc.tensor.matmul(
                                bs[:, col:col + 2],
                                sin_t[h][:, q * P:(q + 1) * P], wband_sb[:],
                                start=True, stop=True,
                            )
                            nc.tensor.matmul(
                                bs[:, col + 2:col + 4],
                                cos_t[h][:, q * P:(q + 1) * P], wband_sb[:],
                                start=True, stop=True,
                            )

                if it == N_STEPS:
                    continue

                # all-DVE update block per stream (no cross-engine round
                # trips): B = sin*u gates t = phi - B (bf16 2x TT); the
                # negated-K products ma = cos*(-v) feed the wrap's
                # subtrahend chunk by chunk: phi' = wrap((t - ma) + dtw)
                for h in horder:
                    mb = work.tile([P, wh], bf16, tag=f"mb{h}", name=f"mb{h}")
                    nc.vector.tensor_tensor(
                        mb[:], sin_t[h][:], vu_a[h][:, 0:wh], ALU.mult)
                    t = work.tile([P, wh], bf16, tag=f"t{h}", name=f"t{h}")
                    nc.gpsimd.tensor_tensor(t[:], phi[h][:], mb[:],
                                            ALU.subtract)
                    ma0 = work.tile([P, bh], bf16, tag=f"ma0{h}",
                                    name=f"ma0{h}")
                    nc.vector.tensor_tensor(
                        ma0[:], cos_t[h][:, 0:bh], vu_a[h][:, wh:wh + bh],
                        ALU.mult)
                    nc.vector._custom_dve(
                        wrap_sub,
                        out=phi[h][:, 0:bh],
                        in0=t[:, 0:bh], in1=ma0[:],
                        s0=dtw_sb[:, 0:1], s1=PI, imm2=TWO_PI,
                    )
                    ma1 = work.tile([P, 2 * bh], bf16, tag=f"ma1{h}",
                                    name=f"ma1{h}")
                    nc.vector.tensor_tensor(
                        ma1[:], cos_t[h][:, bh:3 * bh], vu_b[h][:],
                        ALU.mult)
                    for c0, c1 in ([(1, 3)] if merge_g else [(1, 2), (2, 3)]):
                        nc.vector._custom_dve(
                            wrap_sub,
                            out=phi[h][:, c0 * bh:c1 * bh],
                            in0=t[:, c0 * bh:c1 * bh],
                            in1=ma1[:, (c0 - 1) * bh:(c1 - 1) * bh],
                            s0=dtw_sb[:, c0:c0 + 1], s1=PI, imm2=TWO_PI,
                        )

            # ---- outputs ----
            bs_sb = work.tile([P, N_STEPS * 16], f32, tag="bs_sb", name="bs_sb")
            nc.vector.tensor_copy(bs_sb[:], bs[:])
            nc.sync.dma_start(bs_out[:], bs_sb[:])

    nc.compile()
    return nc


def kernel(x, W_phase, W_amp, omega, K):
    from concourse.bass_utils import run_bass_kernel_spmd

    x = np.asarray(x, dtype=np.float32)
    W_phase = np.asarray(W_phase, dtype=np.float32)
    W_amp = np.asarray(W_amp, dtype=np.float32)
    omega = np.asarray(omega, dtype=np.float32)
    K = np.asarray(K, dtype=np.float32)

    perm = _osc_perm()

    # ---- host-side packing ----
    wpT = np.zeros((N_DIMS, NCH * P), dtype=np.float32)
    waT = np.zeros((N_DIMS, NCH * P), dtype=np.float32)
    dtw = np.zeros((P, NCH), dtype=np.float32)
    for c in range(NCH):
        n = CHUNK_REAL[c]
        idx = perm[c, :n]
        wpT[:, c * P:c * P + n] = W_phase[idx].T
        waT[:, c * P:c * P + n] = W_amp[idx].T
        w = DT * omega[idx].astype(np.float64)
        dtw[:n, c] = (np.mod(w + PI, TWO_PI) - PI).astype(np.float32)

    kT = np.zeros((NCH * P, NCH * P), dtype=np.float32)
    for jc in range(NCH):
        nj = CHUNK_REAL[jc]
        jdx = perm[jc, :nj]
        for ic in range(NCH):
            ni = CHUNK_REAL[ic]
            idx = perm[ic, :ni]
            kT[jc * P:jc * P + nj, ic * P:ic * P + ni] = DT * K[np.ix_(idx, jdx)].T

    nz = [
        (jc, ic)
        for jc in range(NCH)
        for ic in range(NCH)
        if np.any(kT[jc * P:(jc + 1) * P, ic * P:(ic + 1) * P] != 0.0)
    ]
    # every output chunk needs at least one matmul so its PSUM slice is
    # written (zero block is fine)
    for ic in range(NCH):
        if not any(i2 == ic for (_, i2) in nz):
            nz.append((ic, ic))
    nz_pairs = tuple(sorted(nz))

    wband = np.zeros((P, 2), dtype=np.float32)
    wband[:N_DELTA, 0] = 1.0
    wband[N_DELTA:N_DELTA + N_THETA, 1] = 1.0

    merge_g = bool(np.array_equal(dtw[:, 1], dtw[:, 2]))
    key = (nz_pairs, merge_g)
    if key not in _COMPILED:
        _COMPILED[key] = _build_program(nz_pairs, merge_g)
    nc = _COMPILED[key]

    in_maps = []
    for i in range(N_CORES):
        xs = x[i * BL:(i + 1) * BL]
        xst = np.ascontiguousarray(xs.T)
        in_maps.append({
            "xT": xst,
            "wpT": wpT, "waT": waT, "kT": kT, "dtw": dtw, "wband": wband,
        })

    res = run_bass_kernel_spmd(nc, in_maps, core_ids=list(range(N_CORES)))

    # ---- host-side unshard + exact amp reconstruction ----
    band_of = np.zeros(N_TOTAL, dtype=np.int64)
    band_of[N_DELTA:N_DELTA + N_THETA] = 1
    band_of[N_DELTA + N_THETA:] = 2

    out = np.empty((BATCH, N_TOTAL), dtype=np.float32)
    for i in range(N_CORES):
        r = res.results[i]
        amp0v = np.maximum(np.abs(r["amp0"].astype(np.float64)), EPS)
        bsv = r["bsums"].astype(np.float64)
        bss = bsv.reshape(P, N_STEPS, 4, 4)          # [p, step, q, (Sd St Cd Ct)]
        S = bss[:, :, :, 0:2]                        # [p, k, q, band]
        C = bss[:, :, :, 2:4]
        cosm = C / np.sqrt(S * S + C * C)
        f = 1.0 + DT * PAC * cosm                    # [p, k, q, band]
        Pk = np.cumprod(f, axis=1)
        m = np.minimum.accumulate(Pk, axis=1)
        Pn = Pk[:, -1]                               # [p, q, band]
        mn = m[:, -1]
        Pfac = np.ones((BL, 3))
        Efac = np.ones((BL, 3))
        for q in range(4):
            sl = slice(q * P, (q + 1) * P)
            Pfac[sl, 1] = Pn[:, q, 0]
            Pfac[sl, 2] = Pn[:, q, 1]
            Efac[sl, 1] = Pn[:, q, 0] / mn[:, q, 0]
            Efac[sl, 2] = Pn[:, q, 1] / mn[:, q, 1]
        a0 = np.empty((BL, N_TOTAL))
        for c in range(NCH):
            n = CHUNK_REAL[c]
            idx = perm[c, :n]
            a0[:, idx] = amp0v[:n, c * BL:(c + 1) * BL].T
        amp = np.maximum(a0 * Pfac[:, band_of], EPS * Efac[:, band_of])
        out[i * BL:(i + 1) * BL] = amp.astype(np.float32)
    return out


# revision 42
# speedup vs baseline: 1.2160x; 1.2160x over previous
"""Trainium2 Bass kernel for DiscreteDeltaThetaGammaLayer.

Coupled Kuramoto-oscillator recurrence:
  phase0 = (x @ W_phase.T) mod 2pi ; amp0 = max(|x @ W_amp.T|, eps)
  32 steps of: intra-band Kuramoto coupling (phase), PAC amplitude modulation
  output: final amp  (4096, 352) f32

Strategy (8 NeuronCores, data-parallel over batch, 512 rows/core):
  - State held transposed [128 osc partitions x batch free]. Oscillators
    permuted into chunks: c0 = delta(32)+theta(64)+pad(32), c1/c2 = gamma
    halves. Zero blocks of K.T are skipped.
  - Phase kept wrapped in [-pi, pi] (ACT Sin LUT accurate there only);
    cos(phi) = sin(pi/2 - |phi|). The |phi| pass and the t = phi + cos*v
    add run on the Pool engine (all-SBUF; GPSIMD cannot read PSUM), the
    [cos|sin]*[v|u] product and the fused WRAP_SUB phase update
    wrap((t - sin*u) + dt*omega) run on the Vector engine.
  - Per-core batch split into two 256-col streams so the recurrence
    pipelines across engines, and every f32r matmul has >=256 moving cols
    (full PE rate).
  - Band sums (PE matmuls vs band-indicator) write straight into a dedicated
    PSUM bank laid out [step, q, (Sd St Cd Ct)]; one DMA at the end. Host
    reconstructs amp exactly: f_k, prefix products P, running min m,
    amp = max(amp0*P, eps*P/m).
  - Projections are full-core (512 cols, f32r full rate) into PSUM reused by
    the per-stream vu tiles.
"""

import math
import os
import sys

sys.path.insert(0, "/opt/trn_rl_repo")

import numpy as np

# ---- problem constants (module hyperparameters) ----
N_DELTA, N_THETA, N_GAMMA = 32, 64, 256
N_TOTAL = 352
N_DIMS = 1024
BATCH = 4096
N_STEPS = 32
DT = 0.01
PAC = 0.3
EPS = 1e-6
TWO_PI = 2.0 * math.pi
PI = math.pi

N_CORES = 8
BL = BATCH // N_CORES          # 512 batch rows per core
BHS = [256, 256]               # two independent streams
OFFS = [0, 256]
NH = len(BHS)
P = 128
NCH = 3                        # oscillator chunks (3*128 = 384 >= 352)
CHUNK_REAL = [96, 128, 128]
KD = N_DIMS // P               # 8 contraction chunks for the projections

LAST_EXEC_NS = None
_COMPILED = {}
_WRAP_SUB = None


def _osc_perm():
    """orig oscillator index for each (chunk, partition); -1 for pads."""
    perm = -np.ones((NCH, P), dtype=np.int64)
    perm[0, :96] = np.arange(96)           # delta + theta
    perm[1, :] = 96 + np.arange(128)       # gamma 0:128
    perm[2, :] = 224 + np.arange(128)      # gamma 128:256
    return perm


def _get_wrap_sub():
    """Custom DVE op: out = wrap((in0 - in1) + s0) into [-s1, s1], period imm2."""
    global _WRAP_SUB
    if _WRAP_SUB is not None:
        return _WRAP_SUB
    from concourse.dve_spec import C0, C1, C2, Spec, Src0, Src1, lower
    from concourse.dve_uop import DveOpSpec
    import concourse.dve_ops as dvo

    def _ref(in0, in1, s0, s1, imm2):
        y = (in0 - in1) + s0
        return (y + imm2 * (y < -s1).astype(np.float32)
                - imm2 * (y > s1).astype(np.float32)).astype(np.float32)

    _y = (Src0 - Src1) + C0
    spec = Spec(body=(_y + C2 * (_y < -C1)) - C2 * (_y > C1), reference=_ref)
    shas = {}
    for ver in ("v3", "v4"):
        tmp = DveOpSpec(name="WRAP_SUB_KERNEL", opcode=31,
                        uops=lower(spec, ver=ver), rd1_en=True)
        shas[ver] = tmp.sha(ver)
    op = dvo.DveOp("WRAP_SUB_KERNEL", spec, subdim=False, uops_sha=shas)
    dvo.OPS.append(op)
    dvo.CUSTOM_DVE_SPECS[op.name] = op.spec
    dvo._SUB_OPCODE_FOR_NAME[op.name] = dvo._CUSTOM_DVE_ROW_BASE + len(dvo.OPS) - 1
    _WRAP_SUB = op
    return op


def _build_program(nz_pairs, merge_g=False):
    import concourse.bass as bass
    import concourse.tile as tile
    from concourse import bacc, mybir

    wrap_sub = _get_wrap_sub()

    f32 = mybir.dt.float32
    f32r = mybir.dt.float32r
    bf16 = mybir.dt.bfloat16
    AF = mybir.ActivationFunctionType
    ALU = mybir.AluOpType

    nc = bacc.Bacc("TRN2", target_bir_lowering=False, debug=False)

    # ---- DRAM I/O ----
    xT = nc.dram_tensor("xT", [N_DIMS, BL], f32r, kind="ExternalInput").ap()
    wpT = nc.dram_tensor("wpT", [N_DIMS, NCH * P], f32r, kind="ExternalInput").ap()
    waT = nc.dram_tensor("waT", [N_DIMS, NCH * P], f32r, kind="ExternalInput").ap()
    kT = nc.dram_tensor("kT", [NCH * P, NCH * P], f32, kind="ExternalInput").ap()
    dtw = nc.dram_tensor("dtw", [P, NCH], f32, kind="ExternalInput").ap()
    wband = nc.dram_tensor("wband", [P, 2], f32, kind="ExternalInput").ap()

    amp0_out = nc.dram_tensor("amp0", [P, NCH * BL], f32, kind="ExternalOutput").ap()
    # bsums layout: col = step*16 + q*4 + (Sd St Cd Ct), q = batch block (q*128)
    bs_out = nc.dram_tensor(
        "bsums", [P, N_STEPS * 16], f32, kind="ExternalOutput"
    ).ap()

    with tile.TileContext(nc) as tc:
        with (
            tc.tile_pool(name="state", bufs=1) as state_pool,
            tc.tile_pool(name="weights", bufs=1) as wpool,
            tc.tile_pool(name="work", bufs=3) as work,
            tc.tile_pool(name="psum", bufs=1, space="PSUM") as psum,
        ):
            # ---- persistent constants ----
            dtw_sb = wpool.tile([P, NCH], f32, tag="dtw")
            nc.gpsimd.dma_start(dtw_sb[:], dtw[:])
            pihalf = wpool.tile([P, 1], f32, tag="pihalf")
            nc.vector.memset(pihalf[:], PI / 2.0)
            wband_f = wpool.tile([P, 2], f32, tag="wband_f")
            nc.gpsimd.dma_start(wband_f[:], wband[:])
            wband_sb = wpool.tile([P, 2], bf16, tag="wband")
            nc.vector.tensor_copy(wband_sb[:], wband_f[:])

            kt_sb = {}
            kt_ng = {}
            for (jc, ic) in nz_pairs:
                tf = work.tile([P, P], f32, tag="ktld")
                nc.gpsimd.dma_start(tf[:], kT[jc * P:(jc + 1) * P, ic * P:(ic + 1) * P])
                t = wpool.tile([P, P], bf16, tag=f"kt_{jc}_{ic}")
                nc.vector.tensor_copy(t[:], tf[:])
                kt_sb[(jc, ic)] = t
                tn = wpool.tile([P, P], bf16, tag=f"ktn_{jc}_{ic}")
                nc.vector.tensor_scalar_mul(tn[:], tf[:], -1.0)
                kt_ng[(jc, ic)] = tn

            # ---- big input loads ----
            # phase-path first: only it gates the recurrence; the amp
            # projection can lag and overlap the first steps.
            xk = []
            wk_all = {}
            for k in range(KD):
                t = wpool.tile([P, BL], f32r, tag=f"x_{k}")
                eng = nc.gpsimd if k % 2 == 0 else nc.sync
                eng.dma_start(t[:], xT[k * P:(k + 1) * P, :])
                xk.append(t)
                t = wpool.tile([P, NCH * P], f32r, tag=f"w0_{k}")
                nc.sync.dma_start(t[:], wpT[k * P:(k + 1) * P, :])
                wk_all[(0, k)] = t
            for k in range(KD):
                t = wpool.tile([P, NCH * P], f32r, tag=f"w1_{k}")
                nc.sync.dma_start(t[:], waT[k * P:(k + 1) * P, :])
                wk_all[(1, k)] = t

            # ---- per-stream state ----
            # vu halves are separate PSUM tiles so a product TT only waits on
            # its own half's coupling matmuls (dependency tracking is
            # tile-granular)
            WH = NCH * BHS[0]  # 768
            phi = []
            for h in range(NH):
                phi.append(state_pool.tile([P, WH], bf16, tag=f"phi{h}",
                                           name=f"phi{h}"))
            # TILE-A = [u(768) | v_chunk0(256)] (2 banks), TILE-B =
            # [v_c1 | v_c2] (1 bank) per stream: 7 banks total with bs.
            # The critical sin*u product reads TILE-A, whose only
            # sin-dependent writer (the single v_c0 matmul) finishes early.
            # The v half uses negated K so the wrap op (which subtracts its
            # in1) effectively adds cos*v.
            vu_a, vu_b = [], []
            for h in range(NH):
                vu_a.append(psum.tile([P, WH + BHS[0]], f32, tag=f"vua{h}",
                                      name=f"vua{h}"))
                vu_b.append(psum.tile([P, 2 * BHS[0]], f32, tag=f"vub{h}",
                                      name=f"vub{h}"))
            bs = psum.tile([P, N_STEPS * 16], f32, tag="bs", name="bs")

            # ---- initial projections: per (stream, chunk) 256-col pieces
            # laid out to match the per-stream vu tiles (f32r full rate) ----
            # per-stream: phase proj + wrap, then amp proj + abs. Stream 1's
            # projections queue behind stream 0's on the PE, which skews the
            # two recurrences by roughly half a step cycle - that anti-phase
            # is what lets the engine queues interleave the streams.
            ab = work.tile([P, NCH * BL], f32, tag="abs0", name="abs0")
            for h in range(NH):
                bh = BHS[h]
                for c in range(NCH):
                    acc = vu_a[h][:, c * bh:(c + 1) * bh]
                    for k in range(KD):
                        nc.tensor.matmul(
                            acc, wk_all[(0, k)][:, c * P:(c + 1) * P],
                            xk[k][:, OFFS[h]:OFFS[h] + bh],
                            start=(k == 0), stop=(k == KD - 1),
                        )
                nc.vector.add_range_wrap(phi[h][:], vu_a[h][:, 0:WH], 0.0, PI,
                                         TWO_PI)
                adst = [vu_b[h][:, 0:bh], vu_b[h][:, bh:2 * bh],
                        vu_a[h][:, WH:WH + bh]]
                for c in range(NCH):
                    for k in range(KD):
                        nc.tensor.matmul(
                            adst[c], wk_all[(1, k)][:, c * P:(c + 1) * P],
                            xk[k][:, OFFS[h]:OFFS[h] + bh],
                            start=(k == 0), stop=(k == KD - 1),
                        )
                for c in range(NCH):
                    nc.scalar.activation(
                        ab[:, c * BL + OFFS[h]:c * BL + OFFS[h] + bh],
                        adst[c], AF.Abs)
            nc.sync.dma_start(amp0_out[:], ab[:])

            # ---- the recurrence: NH independent streams, staged emission so
            # each engine queue always has the other stream's independent
            # work between dependent ops ----
            spans = [(0, 1), (1, 3)] if merge_g else [(0, 1), (1, 2), (2, 3)]
            for it in range(N_STEPS + 1):
                bh = BHS[0]
                wh = NCH * bh
                nq = bh // P
                cw = bh if it == N_STEPS else wh
                horder = list(range(NH))
                sin_t, cos_t, pa_t = {}, {}, {}
                for h in horder:
                    sin_t[h] = work.tile([P, wh], bf16, tag=f"css{h}",
                                         name=f"css{h}")
                    cos_t[h] = work.tile([P, wh], bf16, tag=f"csc{h}",
                                         name=f"csc{h}")
                    pa_t[h] = work.tile([P, wh], f32, tag=f"pa{h}",
                                        name=f"pa{h}")
                # trig: abs -> cos first (cos gates the u matmuls feeding
                # the critical sin*u product); sin runs while the PE does u
                for h in horder:
                    nc.scalar.activation(pa_t[h][:, 0:cw], phi[h][:, 0:cw],
                                         AF.Abs)
                    nc.scalar.activation(cos_t[h][:, 0:cw], pa_t[h][:, 0:cw],
                                         AF.Sin, bias=pihalf[:], scale=-1.0)
                    nc.scalar.activation(sin_t[h][:, 0:cw], phi[h][:, 0:cw],
                                         AF.Sin)

                # coupling: u = (dt*K) cos (plain) first, then the negated
                # v = -(dt*K) sin, v_c0 first so TILE-A completes early
                if it < N_STEPS:
                    for h in horder:
                        vdst = [vu_a[h][:, WH:WH + bh],
                                vu_b[h][:, 0:bh], vu_b[h][:, bh:2 * bh]]
                        for neg, src in ((False, cos_t[h]), (True, sin_t[h])):
                            for ic in range(NCH):
                                jcs = [jc for (jc, i2) in nz_pairs if i2 == ic]
                                if neg:
                                    dst = vdst[ic]
                                    ktu = kt_ng
                                else:
                                    dst = vu_a[h][:, ic * bh:(ic + 1) * bh]
                                    ktu = kt_sb
                                for n, jc in enumerate(jcs):
                                    nc.tensor.matmul(
                                        dst,
                                        ktu[(jc, ic)][:],
                                        src[:, jc * bh:(jc + 1) * bh],
                                        start=(n == 0),
                                        stop=(n == len(jcs) - 1),
                                    )

                # band sums -> bs[step, q]: (Sd St Cd Ct), q = h*nq + local
                # (off the critical path; emitted after the coupling)
                if it > 0:
                    for h in horder:
                        for q in range(nq):
                            col = (it - 1) * 16 + (h * nq + q) * 4
                            nc.tensor.matmul(
                                bs[:, col:col + 2],
                                sin_t[h][:, q * P:(q + 1) * P], wband_sb[:],
                                start=True, stop=True,
                            )
                            nc.tensor.matmul(
                                bs[:, col + 2:col + 4],
                                cos_t[h][:, q * P:(q + 1) * P], wband_sb[:],
                                start=True, stop=True,
                            )

                if it == N_STEPS:
                    continue

                # all-DVE update block per stream (no cross-engine round
                # trips): B = sin*u gates t = phi - B (bf16 2x TT); the
                # negated-K products ma = cos*(-v) feed the wrap's
                # subtrahend chunk by chunk: phi' = wrap((t - ma) + dtw)
                for h in horder:
                    mb = work.tile([P, wh], bf16, tag=f"mb{h}", name=f"mb{h}")
                    nc.vector.tensor_tensor(
                        mb[:], sin_t[h][:], vu_a[h][:, 0:wh], ALU.mult)
                    t = work.tile([P, wh], bf16, tag=f"t{h}", name=f"t{h}")
                    nc.vector.tensor_tensor(t[:], phi[h][:], mb[:],
                                            ALU.subtract)
                    ma0 = work.tile([P, bh], bf16, tag=f"ma0{h}",
                                    name=f"ma0{h}")
                    nc.vector.tensor_tensor(
                        ma0[:], cos_t[h][:, 0:bh], vu_a[h][:, wh:wh + bh],
                        ALU.mult)
                    nc.vector._custom_dve(
                        wrap_sub,
                        out=phi[h][:, 0:bh],
                        in0=t[:, 0:bh], in1=ma0[:],
                        s0=dtw_sb[:, 0:1], s1=PI, imm2=TWO_PI,
                    )
                    ma1 = work.tile([P, 2 * bh], bf16, tag=f"ma1{h}",
                                    name=f"ma1{h}")
                    nc.vector.tensor_tensor(
                        ma1[:], cos_t[h][:, bh:3 * bh], vu_b[h][:],
                        ALU.mult)
                    for c0, c1 in ([(1, 3)] if merge_g else [(1, 2), (2, 3)]):
                        nc.vector._custom_dve(
                            wrap_sub,
                            out=phi[h][:, c0 * bh:c1 * bh],
                            in0=t[:, c0 * bh:c1 * bh],
                            in1=ma1[:, (c0 - 1) * bh:(c1 - 1) * bh],
                            s0=dtw_sb[:, c0:c0 + 1], s1=PI, imm2=TWO_PI,
                        )

            # ---- outputs ----
            bs_sb = work.tile([P, N_STEPS * 16], f32, tag="bs_sb", name="bs_sb")
            nc.vector.tensor_copy(bs_sb[:], bs[:])
            nc.sync.dma_start(bs_out[:], bs_sb[:])

    nc.compile()
    return nc


def kernel(x, W_phase, W_amp, omega, K):
    from concourse.bass_utils import run_bass_kernel_spmd

    x = np.asarray(x, dtype=np.float32)
    W_phase = np.asarray(W_phase, dtype=np.float32)
    W_amp = np.asarray(W_amp, dtype=np.float32)
    omega = np.asarray(omega, dtype=np.float32)
    K = np.asarray(K, dtype=np.float32)

    perm = _osc_perm()

    # ---- host-side packing ----
    wpT = np.zeros((N_DIMS, NCH * P), dtype=np.float32)
    waT = np.zeros((N_DIMS, NCH * P), dtype=np.float32)
    dtw = np.zeros((P, NCH), dtype=np.float32)
    for c in range(NCH):
        n = CHUNK_REAL[c]
        idx = perm[c, :n]
        wpT[:, c * P:c * P + n] = W_phase[idx].T
        waT[:, c * P:c * P + n] = W_amp[idx].T
        w = DT * omega[idx].astype(np.float64)
        dtw[:n, c] = (np.mod(w + PI, TWO_PI) - PI).astype(np.float32)

    kT = np.zeros((NCH * P, NCH * P), dtype=np.float32)
    for jc in range(NCH):
        nj = CHUNK_REAL[jc]
        jdx = perm[jc, :nj]
        for ic in range(NCH):
            ni = CHUNK_REAL[ic]
            idx = perm[ic, :ni]
            kT[jc * P:jc * P + nj, ic * P:ic * P + ni] = DT * K[np.ix_(idx, jdx)].T

    nz = [
        (jc, ic)
        for jc in range(NCH)
        for ic in range(NCH)
        if np.any(kT[jc * P:(jc + 1) * P, ic * P:(ic + 1) * P] != 0.0)
    ]
    # every output chunk needs at least one matmul so its PSUM slice is
    # written (zero block is fine)
    for ic in range(NCH):
        if not any(i2 == ic for (_, i2) in nz):
            nz.append((ic, ic))
    nz_pairs = tuple(sorted(nz))

    wband = np.zeros((P, 2), dtype=np.float32)
    wband[:N_DELTA, 0] = 1.0
    wband[N_DELTA:N_DELTA + N_THETA, 1] = 1.0

    merge_g = bool(np.array_equal(dtw[:, 1], dtw[:, 2]))
    key = (nz_pairs, merge_g)
    if key not in _COMPILED:
        _COMPILED[key] = _build_program(nz_pairs, merge_g)
    nc = _COMPILED[key]

    in_maps = []
    for i in range(N_CORES):
        xs = x[i * BL:(i + 1) * BL]
        xst = np.ascontiguousarray(xs.T)
        in_maps.append({
            "xT": xst,
            "wpT": wpT, "waT": waT, "kT": kT, "dtw": dtw, "wband": wband,
        })

    res = run_bass_kernel_spmd(nc, in_maps, core_ids=list(range(N_CORES)))

    # ---- host-side unshard + exact amp reconstruction ----
    band_of = np.zeros(N_TOTAL, dtype=np.int64)
    band_of[N_DELTA:N_DELTA + N_THETA] = 1
    band_of[N_DELTA + N_THETA:] = 2

    out = np.empty((BATCH, N_TOTAL), dtype=np.float32)
    for i in range(N_CORES):
        r = res.results[i]
        amp0v = np.maximum(np.abs(r["amp0"].astype(np.float64)), EPS)
        bsv = r["bsums"].astype(np.float64)
        bss = bsv.reshape(P, N_STEPS, 4, 4)          # [p, step, q, (Sd St Cd Ct)]
        S = bss[:, :, :, 0:2]                        # [p, k, q, band]
        C = bss[:, :, :, 2:4]
        cosm = C / np.sqrt(S * S + C * C)
        f = 1.0 + DT * PAC * cosm                    # [p, k, q, band]
        Pk = np.cumprod(f, axis=1)
        m = np.minimum.accumulate(Pk, axis=1)
        Pn = Pk[:, -1]                               # [p, q, band]
        mn = m[:, -1]
        Pfac = np.ones((BL, 3))
        Efac = np.ones((BL, 3))
        for q in range(4):
            sl = slice(q * P, (q + 1) * P)
            Pfac[sl, 1] = Pn[:, q, 0]
            Pfac[sl, 2] = Pn[:, q, 1]
            Efac[sl, 1] = Pn[:, q, 0] / mn[:, q, 0]
            Efac[sl, 2] = Pn[:, q, 1] / mn[:, q, 1]
        a0 = np.empty((BL, N_TOTAL))
        for c in range(NCH):
            n = CHUNK_REAL[c]
            idx = perm[c, :n]
            a0[:, idx] = amp0v[:n, c * BL:(c + 1) * BL].T
        amp = np.maximum(a0 * Pfac[:, band_of], EPS * Efac[:, band_of])
        out[i * BL:(i + 1) * BL] = amp.astype(np.float32)
    return out
